# revision 17
# baseline (speedup 1.0000x reference)
"""Bass/Trainium2 kernel for nn_Blob_DC_and_BCE_loss (loss_fn).

Strategy
--------
The loss decomposes into sums of five per-voxel fields
    f1 = softplus(x) - x*y,  p = sigmoid(x),  p*y,  y,  1
over (a) the full volumes (global dice/BCE + per-sample fallback) and
(b) per-target-component "keep" masks
    keep_c(v) = (t(v) in {0,c}) & (m(v) in {0,c})
where t = target CC rank and m = rank of the target component each
predicted CC maps to (max-overlap label).  Since keep_c == 1 outside a
small neighbourhood of the lesions, the masked sums equal
(global sums - ROI sums) + ROI keep_c sums, with the ROI a set of 32^3
boxes around the target components.

Work split:
  host   - CC labeling (tiny fraction of runtime), box/ownership setup,
           final O(1) scalar assembly
  device - all O(N) math: 8-way D-slab data-parallel global reductions,
           one ROI box per core for the masked per-label reductions.
           Work is spread across ACT (exp/ln/sigmoid), DVE (fused
           multiply-reduce) and GPSIMD (masks, plain sums) engines.

sigmoid is computed as exp(x - softplus(x)) so every ACT op lives in the
single Exp+Ln activation table (no per-op table reloads), and softplus
is evaluated on its small branch for accuracy.
"""

import math
import os

import numpy as np

B = 2
D = H = W = 128
N = D * H * W
NCORES = 8
SLAB = D // NCORES            # 16 depth slices per core
GFD = SLAB * H * W // 128     # 2048: free dim of one sample slab tile
BOX = 32                      # ROI box edge
BFD = BOX ** 3 // 128         # 256: free dim of one box tile
SENT = 9.0                    # sentinel rank for non-owned ROI voxels
K_DEV = 4                     # labels per sample handled on device
LOG2 = math.log(2.0)
SMOOTH = 1e-5

# global-sum output columns (per sample): softplus(-x), x, x*y, p, p*y, y
GCOLS = 6
GCH = 2   # global-phase chunks per sample slab
# ROI output columns.
# fast variant (every box holds a single component rank):
#   own {f1,p,py,y,cnt} + ownbg {f1,p,py,y,cnt}   (bg = owned & t==0 & m==0)
# general variant: own {...} + 4 x keep_c {...}
RCOLS_FAST = 10
RCOLS = 5 * (1 + K_DEV)


# --------------------------------------------------------------------------
# host-side connected components (scipy if present, numpy fallback)
# --------------------------------------------------------------------------

def _label_np(mask):
    """6-connectivity CC labeling, pure numpy (iterative min-propagation)."""
    lab = np.where(mask, np.arange(1, mask.size + 1, dtype=np.int64
                                   ).reshape(mask.shape), 0)
    while True:
        new = lab.copy()
        sl = new[1:, :, :]; np.minimum(sl, np.where(lab[:-1] > 0, lab[:-1], sl), out=sl)
        sl = new[:-1, :, :]; np.minimum(sl, np.where(lab[1:] > 0, lab[1:], sl), out=sl)
        sl = new[:, 1:, :]; np.minimum(sl, np.where(lab[:, :-1] > 0, lab[:, :-1], sl), out=sl)
        sl = new[:, :-1, :]; np.minimum(sl, np.where(lab[:, 1:] > 0, lab[:, 1:], sl), out=sl)
        sl = new[:, :, 1:]; np.minimum(sl, np.where(lab[:, :, :-1] > 0, lab[:, :, :-1], sl), out=sl)
        sl = new[:, :, :-1]; np.minimum(sl, np.where(lab[:, :, 1:] > 0, lab[:, :, 1:], sl), out=sl)
        new = np.where(mask, new, 0)
        if np.array_equal(new, lab):
            break
        lab = new
    uniq = np.unique(lab[lab > 0])
    remap = np.zeros(int(lab.max()) + 1, np.int64)
    remap[uniq] = np.arange(1, len(uniq) + 1)
    return remap[lab], len(uniq)


def _cc_label(mask):
    try:
        from scipy import ndimage as ndi
        st = ndi.generate_binary_structure(3, 1)
        lab, n = ndi.label(mask, structure=st)
        return lab.astype(np.int64), int(n)
    except Exception:
        return _label_np(mask)


def _host_metadata(x, y):
    """Per-sample rank volumes t8/m8 and component counts."""
    meta = []
    lin1 = np.arange(1, N + 1, dtype=np.int64).reshape(D, H, W)
    for b in range(B):
        tgt = y[b, 0] > 0.5
        pred = x[b, 0] >= 0.0
        tlab, ntc = _cc_label(tgt)
        plab, npc = _cc_label(pred)
        # reference label value = max linear index + 1 within target comp
        tmax = np.zeros(ntc + 1, np.int64)
        np.maximum.at(tmax, tlab.ravel(), np.where(tgt, lin1, 0).ravel())
        tval = np.where(tgt, tmax[tlab], 0)
        # map each predicted comp to the max target label it overlaps
        pmax = np.zeros(npc + 1, np.int64)
        np.maximum.at(pmax, plab.ravel(), tval.ravel())
        mval = np.where(pred, pmax[plab], 0)
        # ranks: descending reference label order (top_k order)
        labels_desc = np.sort(np.unique(tval[tval > 0]))[::-1]
        n_cc = len(labels_desc)
        assert n_cc <= K_DEV, f"sample {b}: {n_cc} comps > {K_DEV} unsupported"
        rank_of = np.zeros(int(tval.max()) + 1 if n_cc else 1, np.int64)
        for i, L in enumerate(labels_desc):
            rank_of[L] = i + 1
        t8 = rank_of[tval].astype(np.float32)
        m8 = rank_of[mval].astype(np.float32)
        meta.append(dict(t8=t8, m8=m8, n_cc=n_cc))
    return meta


def _build_boxes(meta):
    """Cover the interesting voxels with <= NCORES boxes of BOX^3.

    Each connected cluster of the interesting set (target comp + its
    matched predicted comps) is covered by a grid of boxes over its bbox.
    Returns list of (sample, d0, h0, w0) and per-sample ownership arrays
    (box index owning each voxel, -1 if none).
    """
    boxes = []
    owners = []
    for b in range(B):
        t8, m8 = meta[b]["t8"], meta[b]["m8"]
        interesting = (t8 > 0) | (m8 > 0)
        own = np.full((D, H, W), -1, np.int32)
        owners.append(own)
        if not interesting.any():
            continue
        clab, ncl = _cc_label(interesting)
        sample_boxes = []
        for ci in range(1, ncl + 1):
            idx = np.argwhere(clab == ci)
            lo, hi = idx.min(axis=0), idx.max(axis=0)  # inclusive
            starts_per_dim = []
            for ax in range(3):
                ext = int(hi[ax] - lo[ax] + 1)
                nb = (ext + BOX - 1) // BOX
                if nb == 1:
                    s0 = int(lo[ax]) - (BOX - ext) // 2
                    starts_per_dim.append([min(max(s0, 0), D - BOX)])
                else:
                    step = (ext - BOX) / (nb - 1)
                    starts_per_dim.append(
                        [min(max(int(lo[ax] + round(i * step)), 0), D - BOX)
                         for i in range(nb)])
            for sd in starts_per_dim[0]:
                for sh in starts_per_dim[1]:
                    for sw in starts_per_dim[2]:
                        bi = len(boxes)
                        assert bi < NCORES, "ROI cover needs > NCORES boxes"
                        boxes.append((b, sd, sh, sw))
                        sample_boxes.append((bi, ci, sd, sh, sw))
                        # interesting voxels of THIS cluster claim the box
                        sl = (slice(sd, sd + BOX), slice(sh, sh + BOX),
                              slice(sw, sw + BOX))
                        region = own[sl]
                        region[(clab[sl] == ci) & (region < 0)] = bi
        # background (non-interesting) voxels: first covering box wins
        for bi, ci, sd, sh, sw in sample_boxes:
            sl = (slice(sd, sd + BOX), slice(sh, sh + BOX),
                  slice(sw, sw + BOX))
            region = own[sl]
            region[region < 0] = bi
    for b in range(B):
        t8, m8 = meta[b]["t8"], meta[b]["m8"]
        assert not (((t8 > 0) | (m8 > 0)) & (owners[b] < 0)).any()
    return boxes, owners


def _build_in_maps(x, y, meta, boxes, owners):
    in_maps = []
    zero_box = np.zeros((128, BFD), np.float32)
    sent_box = np.full((128, BFD), SENT, np.float32)
    for i in range(NCORES):
        d0 = i * SLAB
        gxs = np.stack([x[s, 0, d0:d0 + SLAB].reshape(128, GFD) for s in range(B)])
        gys = np.stack([y[s, 0, d0:d0 + SLAB].reshape(128, GFD) for s in range(B)])
        if i < len(boxes):
            bsmp, bd, bh, bw = boxes[i]
            sl = (slice(bd, bd + BOX), slice(bh, bh + BOX), slice(bw, bw + BOX))
            owned = owners[bsmp][sl] == i
            rxv = np.ascontiguousarray(x[bsmp, 0][sl].reshape(128, BFD))
            ryv = np.ascontiguousarray(y[bsmp, 0][sl].reshape(128, BFD))
            rtv = np.where(owned, meta[bsmp]["t8"][sl], SENT).astype(np.float32).reshape(128, BFD)
            rmv = np.where(owned, meta[bsmp]["m8"][sl], SENT).astype(np.float32).reshape(128, BFD)
        else:
            rxv, ryv, rtv, rmv = zero_box, zero_box, sent_box, sent_box
        in_maps.append(dict(gx=np.ascontiguousarray(gxs), gy=np.ascontiguousarray(gys),
                            rx=rxv, ry=ryv, rt=np.ascontiguousarray(rtv),
                            rm=np.ascontiguousarray(rmv)))
    return in_maps


# --------------------------------------------------------------------------
# device kernel
# --------------------------------------------------------------------------

_BASS = {}


def _build_bass(fast, do_global=True, do_roi=True):
    import itertools

    import concourse.bacc as bacc
    import concourse.tile as tile
    from concourse import mybir

    f32 = mybir.dt.float32
    Alu = mybir.AluOpType
    Act = mybir.ActivationFunctionType
    AX = mybir.AxisListType.X

    rcols = RCOLS_FAST if fast else RCOLS

    nc = bacc.Bacc("TRN2", target_bir_lowering=False)
    gx = nc.dram_tensor("gx", [B, 128, GFD], f32, kind="ExternalInput")
    gy = nc.dram_tensor("gy", [B, 128, GFD], f32, kind="ExternalInput")
    rx = nc.dram_tensor("rx", [128, BFD], f32, kind="ExternalInput")
    ry = nc.dram_tensor("ry", [128, BFD], f32, kind="ExternalInput")
    rt = nc.dram_tensor("rt", [128, BFD], f32, kind="ExternalInput")
    rm = nc.dram_tensor("rm", [128, BFD], f32, kind="ExternalInput")
    og = nc.dram_tensor("og", [128, B * GCH * GCOLS], f32, kind="ExternalOutput")
    orr = nc.dram_tensor("orr", [128, rcols], f32, kind="ExternalOutput")

    with tile.TileContext(nc) as tc:
        with tc.tile_pool(name="acc", bufs=80) as apool, \
             tc.tile_pool(name="gbig", bufs=1) as gpool, \
             tc.tile_pool(name="roi", bufs=1) as rpool:

            _ctr = itertools.count()

            def new_acc():
                return apool.tile([128, 1], f32, tag="acc",
                                  name=f"acc{next(_ctr)}")

            # ---------------- global phase: per-sample slab sums ----------
            # ACT: e = exp(-x); ln = ln(1+e) = softplus(-x) [accum SPM];
            #      sg = exp(-ln) = sigmoid(x) [accum P]; copy x [accum X]
            # DVE: sum x*y, sum sg*y (fused STT), sum y (reduce)
            CH = GCH                     # chunks per sample slab
            CFD = GFD // CH
            for s in (range(B) if do_global else ()):
                for h in range(CH):
                    c0 = h * CFD
                    xt = gpool.tile([128, CFD], f32, tag="xt", bufs=3)
                    yt = gpool.tile([128, CFD], f32, tag="yt", bufs=3)
                    nc.sync.dma_start(xt[:, :], gx[s, :, c0:c0 + CFD])
                    nc.sync.dma_start(yt[:, :], gy[s, :, c0:c0 + CFD])

                    # e = exp(x); sp = ln(1+e) = softplus(x) [accum SP];
                    # sg = exp(x - sp) = sigmoid(x) [accum P]
                    e = gpool.tile([128, CFD], f32, tag="e", bufs=3)
                    nc.scalar.activation(e[:, :], xt[:, :], Act.Exp)
                    sp = gpool.tile([128, CFD], f32, tag="sp", bufs=3)
                    a_sp = new_acc()
                    nc.scalar.activation(sp[:, :], e[:, :], Act.Ln, bias=1.0,
                                         accum_out=a_sp[:, :])
                    nc.sync.dma_start(
                        og[:, (s * CH + h) * GCOLS + 0:
                           (s * CH + h) * GCOLS + 1], a_sp[:, :])
                    xms = gpool.tile([128, CFD], f32, tag="xms", bufs=3)
                    nc.gpsimd.tensor_tensor(xms[:, :], xt[:, :], sp[:, :],
                                            Alu.subtract)
                    sg = gpool.tile([128, CFD], f32, tag="sg", bufs=3)
                    a_p = new_acc()
                    nc.scalar.activation(sg[:, :], xms[:, :], Act.Exp,
                                         accum_out=a_p[:, :])
                    nc.sync.dma_start(
                        og[:, (s * CH + h) * GCOLS + 3:
                           (s * CH + h) * GCOLS + 4], a_p[:, :])

                    sc = gpool.tile([128, CFD], f32, tag="sc", bufs=3)
                    a_xy = new_acc()
                    nc.vector.scalar_tensor_tensor(sc[:, :], xt[:, :], 1.0,
                                                   yt[:, :], Alu.mult, Alu.mult,
                                                   accum_out=a_xy[:, :])
                    nc.sync.dma_start(
                        og[:, (s * CH + h) * GCOLS + 2:
                           (s * CH + h) * GCOLS + 3], a_xy[:, :])
                    sc2 = gpool.tile([128, CFD], f32, tag="sc2", bufs=3)
                    a_i = new_acc()
                    nc.vector.scalar_tensor_tensor(sc2[:, :], sg[:, :], 1.0,
                                                   yt[:, :], Alu.mult, Alu.mult,
                                                   accum_out=a_i[:, :])
                    nc.sync.dma_start(
                        og[:, (s * CH + h) * GCOLS + 4:
                           (s * CH + h) * GCOLS + 5], a_i[:, :])
                    a_g = new_acc()
                    nc.vector.tensor_reduce(a_g[:, :], yt[:, :], AX, Alu.add)
                    nc.sync.dma_start(
                        og[:, (s * CH + h) * GCOLS + 5:
                           (s * CH + h) * GCOLS + 6], a_g[:, :])

            # ---------------- ROI phase: one box per core -----------------
            xr = rpool.tile([128, BFD], f32, tag="xr")
            yr = rpool.tile([128, BFD], f32, tag="yr")
            tr = rpool.tile([128, BFD], f32, tag="tr")
            mr = rpool.tile([128, BFD], f32, tag="mr")
            nc.sync.dma_start(xr[:, :], rx[:, :])
            nc.sync.dma_start(yr[:, :], ry[:, :])
            nc.sync.dma_start(tr[:, :], rt[:, :])
            nc.sync.dma_start(mr[:, :], rm[:, :])

            er = rpool.tile([128, BFD], f32, tag="er")
            nc.scalar.activation(er[:, :], xr[:, :], Act.Exp)
            lr = rpool.tile([128, BFD], f32, tag="lr")
            nc.scalar.activation(lr[:, :], er[:, :], Act.Ln, bias=1.0)
            xmsr = rpool.tile([128, BFD], f32, tag="xmsr")
            nc.gpsimd.tensor_tensor(xmsr[:, :], xr[:, :], lr[:, :],
                                    Alu.subtract)
            pr = rpool.tile([128, BFD], f32, tag="pr")
            nc.scalar.activation(pr[:, :], xmsr[:, :], Act.Exp)

            # f1 = softplus(x) - x*y
            xy = rpool.tile([128, BFD], f32, tag="xy")
            nc.vector.scalar_tensor_tensor(xy[:, :], yr[:, :], 1.0, xr[:, :],
                                           Alu.mult, Alu.mult)
            f1 = rpool.tile([128, BFD], f32, tag="f1")
            nc.gpsimd.tensor_tensor(f1[:, :], lr[:, :], xy[:, :], Alu.subtract)
            pyr = rpool.tile([128, BFD], f32, tag="pyr")
            nc.gpsimd.tensor_tensor(pyr[:, :], pr[:, :], yr[:, :], Alu.mult)

            t0 = rpool.tile([128, BFD], f32, tag="t0")
            nc.vector.tensor_scalar(t0[:, :], tr[:, :], 0.0, None, Alu.is_equal)
            m0 = rpool.tile([128, BFD], f32, tag="m0")
            nc.vector.tensor_scalar(m0[:, :], mr[:, :], 0.0, None, Alu.is_equal)
            own = rpool.tile([128, BFD], f32, tag="own")
            nc.vector.tensor_scalar(own[:, :], tr[:, :], 8.5, None, Alu.is_lt)

            fields = [f1, pr, pyr, yr]

            def mask_sums(mask_tile, colbase):
                for j, ft in enumerate(fields):
                    scr = rpool.tile([128, BFD], f32, tag="scr", bufs=2)
                    a = new_acc()
                    nc.vector.scalar_tensor_tensor(
                        scr[:, :], mask_tile[:, :], 1.0, ft[:, :],
                        Alu.mult, Alu.mult, accum_out=a[:, :])
                    nc.sync.dma_start(orr[:, colbase + j: colbase + j + 1], a[:, :])
                a = new_acc()
                nc.vector.tensor_reduce(a[:, :], mask_tile[:, :], AX, Alu.add)
                nc.sync.dma_start(orr[:, colbase + 4: colbase + 5], a[:, :])

            mask_sums(own, 0)

            if fast:
                # single-rank boxes: keep_c == own for the box rank and
                # own & t==0 & m==0 for every other rank
                g0 = rpool.tile([128, BFD], f32, tag="g0")
                nc.gpsimd.tensor_tensor(g0[:, :], t0[:, :], m0[:, :], Alu.mult)
                bg = rpool.tile([128, BFD], f32, tag="bg")
                nc.gpsimd.tensor_tensor(bg[:, :], own[:, :], g0[:, :], Alu.mult)
                mask_sums(bg, 5)
            else:
                for c in range(1, K_DEV + 1):
                    ta = rpool.tile([128, BFD], f32, tag="ta", bufs=2)
                    nc.vector.scalar_tensor_tensor(ta[:, :], tr[:, :], float(c),
                                                   t0[:, :], Alu.is_equal,
                                                   Alu.logical_or)
                    ma = rpool.tile([128, BFD], f32, tag="ma", bufs=2)
                    nc.vector.scalar_tensor_tensor(ma[:, :], mr[:, :], float(c),
                                                   m0[:, :], Alu.is_equal,
                                                   Alu.logical_or)
                    k = rpool.tile([128, BFD], f32, tag="k", bufs=2)
                    nc.gpsimd.tensor_tensor(k[:, :], ta[:, :], ma[:, :], Alu.mult)
                    mask_sums(k, 5 * c)

    # all our activations (Exp/Ln/Copy) live in one table; hide the other
    # tables from the act-table-load pass so it emits a single load instead
    # of ping-ponging between per-function tables (keeps act_func_set_id
    # indices aligned with act_info.json by preserving dict order)
    import concourse.bacc as _bacc_mod
    _orig_tables = _bacc_mod.get_activation_tables
    _KEEP = "natural_log_exp_and_others"

    def _only_lnexp(arch):
        tabs = _orig_tables(arch)
        assert _KEEP in tabs
        return {name: (funcs if name == _KEEP else set())
                for name, funcs in tabs.items()}

    _bacc_mod.get_activation_tables = _only_lnexp
    try:
        nc.compile()
    finally:
        _bacc_mod.get_activation_tables = _orig_tables
    return nc


def _device_partials_np(in_maps, fast):
    """Numpy mirror of the bass kernel, for pipeline validation."""
    outs = []
    for m in in_maps:
        og = np.zeros((128, B * GCH * GCOLS), np.float32)
        cfd = GFD // GCH
        for s in range(B):
            for h in range(GCH):
                x = m["gx"][s][:, h * cfd:(h + 1) * cfd].astype(np.float64)
                y = m["gy"][s][:, h * cfd:(h + 1) * cfd].astype(np.float64)
                base = (s * GCH + h) * GCOLS
                og[:, base + 0] = np.logaddexp(0, x).sum(1)
                og[:, base + 2] = (x * y).sum(1)
                p = 1.0 / (1.0 + np.exp(-x))
                og[:, base + 3] = p.sum(1)
                og[:, base + 4] = (p * y).sum(1)
                og[:, base + 5] = y.sum(1)
        xr = m["rx"].astype(np.float64); yr = m["ry"].astype(np.float64)
        tr = m["rt"]; mr = m["rm"]
        er = np.exp(-xr)
        f1 = np.log1p(er) + xr * (1 - yr)
        pr = 1.0 / (1.0 + er)
        fields = [f1, pr, pr * yr, yr]
        orr = np.zeros((128, RCOLS_FAST if fast else RCOLS), np.float32)

        def msums(mask, colbase):
            mask = mask.astype(np.float64)
            for j, ft in enumerate(fields):
                orr[:, colbase + j] = (mask * ft).sum(1)
            orr[:, colbase + 4] = mask.sum(1)

        own = tr < 8.5
        msums(own, 0)
        if fast:
            msums(own & (tr == 0) & (mr == 0), 5)
        else:
            for c in range(1, K_DEV + 1):
                k = ((tr == 0) | (tr == c)) & ((mr == 0) | (mr == c))
                msums(k, 5 * c)
        outs.append(dict(og=og, orr=orr))
    return outs


def _device_partials(in_maps, fast):
    if os.environ.get("BLOB_KERNEL_NP"):
        return _device_partials_np(in_maps, fast)
    if fast not in _BASS:
        _BASS[fast] = _build_bass(fast)
    from concourse.bass_utils import run_bass_kernel_spmd
    res = run_bass_kernel_spmd(_BASS[fast], in_maps, core_ids=list(range(NCORES)))
    return res.results


def _box_ranks(meta, boxes, owners):
    """Per box: set of component ranks present among its owned voxels."""
    ranks = []
    for i, (bsmp, bd, bh, bw) in enumerate(boxes):
        sl = (slice(bd, bd + BOX), slice(bh, bh + BOX), slice(bw, bw + BOX))
        owned = owners[bsmp][sl] == i
        t = meta[bsmp]["t8"][sl][owned]
        m = meta[bsmp]["m8"][sl][owned]
        rs = set(np.unique(t[t > 0]).tolist()) | set(np.unique(m[m > 0]).tolist())
        ranks.append({int(r) for r in rs})
    return ranks


# --------------------------------------------------------------------------
# public entry
# --------------------------------------------------------------------------

def kernel(net_output, target):
    x = np.ascontiguousarray(np.asarray(net_output, dtype=np.float32))
    y = np.ascontiguousarray(np.asarray(target, dtype=np.float32))
    assert x.shape == (B, 1, D, H, W) and y.shape == x.shape

    meta = _host_metadata(x, y)
    boxes, owners = _build_boxes(meta)
    ranks = _box_ranks(meta, boxes, owners)
    fast = all(len(r) <= 1 for r in ranks)
    in_maps = _build_in_maps(x, y, meta, boxes, owners)
    results = _device_partials(in_maps, fast)

    # ------------------------ host assembly (O(1)) ------------------------
    og = np.zeros(B * GCH * GCOLS, np.float64)
    for r in results:
        og += np.asarray(r["og"], np.float64).sum(axis=0)
    og = og.reshape(B, GCH, GCOLS).sum(axis=1)
    glob = []
    for s in range(B):
        SP, _, XY, P, I, G = og[s]
        glob.append(dict(f1=SP - XY, p=P, py=I, y=G, cnt=float(N)))

    names = ["f1", "p", "py", "y", "cnt"]
    zero = lambda: dict(f1=0.0, p=0.0, py=0.0, y=0.0, cnt=0.0)
    # K[s][c] - R[s] summed over boxes of sample s (masked-sum correction)
    corr = [[zero() for _ in range(K_DEV + 1)] for _ in range(B)]
    for i in range(len(boxes)):
        bsmp = boxes[i][0]
        part = np.asarray(results[i]["orr"], np.float64).sum(axis=0)
        ownp = part[0:5]
        for c in range(1, K_DEV + 1):
            if fast:
                kp = ownp if (ranks[i] and c in ranks[i]) else part[5:10]
            else:
                kp = part[5 * c: 5 * c + 5]
            for j, nm in enumerate(names):
                corr[bsmp][c][nm] += kp[j] - ownp[j]

    total_contrib = 0.0
    total_count = 0.0
    for s in range(B):
        n_cc = meta[s]["n_cc"]
        g = glob[s]
        if n_cc > 1:
            contrib = 0.0
            for c in range(1, n_cc + 1):
                Sf = {nm: g[nm] + corr[s][c][nm] for nm in names}
                nk = Sf["cnt"]
                bce = (Sf["f1"] + LOG2 * (N - nk)) / N
                Pc = Sf["p"] + 0.5 * (N - nk)
                dc = (2.0 * Sf["py"] + SMOOTH) / max(Pc + Sf["y"] + SMOOTH, 1e-8)
                contrib += bce - dc
            total_contrib += contrib
            total_count += n_cc
        else:
            bce = g["f1"] / N
            dc = (2.0 * g["py"] + SMOOTH) / max(g["p"] + g["y"] + SMOOTH, 1e-8)
            total_contrib += bce - dc
            total_count += 1

    f1b = sum(gl["f1"] for gl in glob)
    bce_g = f1b / (B * N)
    Ib = sum(gl["py"] for gl in glob)
    Pb = sum(gl["p"] for gl in glob)
    Gb = sum(gl["y"] for gl in glob)
    dc_g = (2.0 * Ib + SMOOTH) / max(Pb + Gb + SMOOTH, 1e-8)
    global_loss = bce_g - dc_g

    blob = total_contrib / max(total_count, 1.0)
    out = 0.3 * global_loss + 0.7 * blob
    return np.asarray(out, dtype=np.float32)
